# revision 12
# baseline (speedup 1.0000x reference)
"""Cosformer multi-head attention kernel for 8 Trainium2 NeuronCores.

Reference computation (per batch b):
    q = relu(x @ Wq.T); k = relu(x @ Wk.T); v = x @ Wv.T
    q_ = [q*sin, q*cos]; k_ = [k*sin, k*cos]      (sin/cos indexed by position n)
    kv = k_.T @ v;  z = 1/clip(q_ @ sum_n k_, 1e-6)
    out = (q_ @ kv) * z   followed by a head-scrambling reshape.

Sharding: 8 cores = 4 batches x 2 halves of the output columns (m). Each core
computes q and k in full for its batch (needed for the full-feature contraction)
and v / kv / out only for its 256-column half. No collectives needed.

Decomposition used on-chip (avoids materializing the 2D-wide q_/k_):
    kv_s = (k*sin).T @ v ; kv_c = (k*cos).T @ v            [512, 256] each
    ksum_s = sum_n (k*sin) ; ksum_c = sum_n (k*cos)        [512]
    o_s = q @ kv_s ; o_c = q @ kv_c ; qs_s = q @ ksum_s ; qs_c = q @ ksum_c
    out = (sin*o_s + cos*o_c) / clip(sin*qs_s + cos*qs_c, 1e-6)

All matmuls run as float32r (FP22 multiplies, fp32 accumulate) which is 4x
faster than true fp32 on the PE array at free-dim >= 256. The walrus verifier
requires every tensor consumed by an FP32r matmul to be declared float32r at
its producer, so all matmul operands are float32r end-to-end (bit-compatible
with fp32 on the host side).

The final head-scramble permutation is folded into the output DMA access
pattern; the host only does cheap reshapes to reassemble the full output.
"""

import math
import numpy as np
import ml_dtypes

import concourse.bass as bass
import concourse.mybir as mybir
import concourse.tile as tile
from concourse import bacc
from concourse.bass import ts, ds
from concourse.bass_utils import run_bass_kernel_spmd

B, N, D = 4, 4096, 512
MH = 256          # per-core output column half width
NT = N // 128     # 32 n-tiles
DT = D // 128     # 4 d-tiles
F32 = mybir.dt.float32
F32R = mybir.dt.float32r
BF16 = mybir.dt.bfloat16
import os
MM_DT = BF16 if os.environ.get("KERNEL_BF16") else F32R
AF = mybir.ActivationFunctionType
ALU = mybir.AluOpType


def f32(ap):
    """Read an f32r tensor as plain fp32 (same bits) for non-matmul ops."""
    return ap.bitcast(F32)


def build_program():
    nc = bacc.Bacc("TRN2", target_bir_lowering=False, debug=False, num_devices=8)

    xT = nc.dram_tensor("xT", [D, N], MM_DT, kind="ExternalInput").ap()
    wqT = nc.dram_tensor("wqT", [D, D], MM_DT, kind="ExternalInput").ap()
    wkT = nc.dram_tensor("wkT", [D, D], MM_DT, kind="ExternalInput").ap()
    wvT = nc.dram_tensor("wvT", [D, MH], MM_DT, kind="ExternalInput").ap()
    sct = nc.dram_tensor("sct", [128, 2 * NT], MM_DT, kind="ExternalInput").ap()
    sctf = nc.dram_tensor("sctf", [128, 2 * NT], F32, kind="ExternalInput").ap()
    ident = nc.dram_tensor("ident", [128, 128], MM_DT, kind="ExternalInput").ap()
    out = nc.dram_tensor("out", [N // 2, D], F32, kind="ExternalOutput").ap()

    # scrambled output view: tile rows rr = h*512 + s, cols cc = jj*64 + hd
    # land at out[s*4 + jj, h*64 + hd]
    out_r = out.rearrange("(s j) (h hd) -> s j h hd", j=4, hd=64)

    with tile.TileContext(nc) as tc:
        with (
            tc.tile_pool(name="consts", bufs=1) as consts,
            tc.tile_pool(name="xpool", bufs=1) as xpool,
            tc.tile_pool(name="qpool", bufs=1) as qpool,
            tc.tile_pool(name="work", bufs=2) as work,
            tc.tile_pool(name="opool", bufs=3) as opool,
            tc.tile_pool(name="ppool", bufs=3, space="PSUM") as ppool,
            tc.tile_pool(name="kvpool", bufs=1, space="PSUM") as kvpool,
        ):
            # ---- constants ----
            # Small/weight loads go on the ACT HWDGE ring, x chunks on the SP
            # ring, so the first k/v matmuls (need sct+wk+wv+xc0) start ASAP.
            sct_sb = consts.tile([128, 2 * NT], MM_DT)
            nc.scalar.dma_start(sct_sb, sct)
            sctf_sb = consts.tile([128, 2 * NT], F32)
            nc.scalar.dma_start(sctf_sb, sctf)
            wk_sb = consts.tile([128, DT, D], MM_DT)
            nc.scalar.dma_start(wk_sb, wkT.rearrange("(dt p) m -> p dt m", p=128))
            wv_sb = consts.tile([128, DT, MH], MM_DT)
            nc.scalar.dma_start(wv_sb, wvT.rearrange("(dt p) m -> p dt m", p=128))

            # ---- x, 8 chunks of [128, 4, 512] (1 MiB DMAs) ----
            xc = []
            xT_r = xT.rearrange("(dt p) n -> p dt n", p=128)
            for c in range(8):
                t_ = xpool.tile([128, DT, 512], MM_DT, tag=f"xc{c}", name=f"xc{c}")
                nc.sync.dma_start(t_, xT_r[:, :, ts(c, 512)])
                xc.append(t_)

            wq_sb = consts.tile([128, DT, D], MM_DT)
            nc.scalar.dma_start(wq_sb, wqT.rearrange("(dt p) m -> p dt m", p=128))
            id_sb = consts.tile([128, 128], MM_DT)
            nc.scalar.dma_start(id_sb, ident)

            # qT storage [m-part, mt, n] chunks
            qc = [
                qpool.tile([128, DT, 512], MM_DT, tag=f"qc{c}", name=f"qc{c}")
                for c in range(8)
            ]

            # persistent psum accumulators
            kv_ps = [
                kvpool.tile([128, 512], F32, tag=f"kv{d2}", name=f"kv{d2}")
                for d2 in range(DT)
            ]
            ksum_ps = kvpool.tile([2, 512], F32, tag="ksum", name="ksum")

            # ---- fused loop: k/v/q projections + kv/ksum accumulation ----
            for t in range(NT):
                xt = xc[t // 4]
                nslc = ts(t % 4, 128)  # n-tile slice inside the x chunk
                sin_ap = sctf_sb[:, 2 * t : 2 * t + 1]
                cos_ap = sctf_sb[:, 2 * t + 1 : 2 * t + 2]

                k_ps = ppool.tile([128, 512], F32, tag="proj", name=f"k_ps{t}")
                for dt in range(DT):
                    nc.tensor.matmul(
                        k_ps, xt[:, dt, nslc], wk_sb[:, dt, :],
                        start=dt == 0, stop=dt == DT - 1,
                    )
                v_ps = ppool.tile([128, 512], F32, tag="proj", name=f"v_ps{t}")
                for dt in range(DT):
                    nc.tensor.matmul(
                        v_ps[:, :MH], xt[:, dt, nslc], wv_sb[:, dt, :],
                        start=dt == 0, stop=dt == DT - 1,
                    )
                mt, nc2 = t % 4, t // 4
                q_ps = ppool.tile([128, 512], F32, tag="proj", name=f"q_ps{t}")
                for dt in range(DT):
                    nc.tensor.matmul(
                        q_ps, wq_sb[:, dt, ts(mt, 128)], xc[nc2][:, dt, :],
                        start=dt == 0, stop=dt == DT - 1,
                    )
                nc.scalar.activation(qc[nc2][:, mt, :], q_ps, AF.Relu)

                k_s = work.tile([128, 512], MM_DT, tag="ks", name=f"ks{t}")
                nc.scalar.activation(k_s, k_ps, AF.Relu, scale=sin_ap)
                k_c = work.tile([128, 512], MM_DT, tag="kc", name=f"kc{t}")
                nc.scalar.activation(k_c, k_ps, AF.Relu, scale=cos_ap)
                k_r = work.tile([128, 512], MM_DT, tag="kr", name=f"kr{t}")
                nc.vector.tensor_scalar_max(k_r, k_ps, 0.0)
                v_sb = work.tile([128, MH], MM_DT, tag="vs", name=f"vs{t}")
                nc.vector.tensor_copy(v_sb, v_ps[:, :MH])

                # ksum: [2, 512] += sincos[128,2].T @ relu(k)[128,512]
                nc.tensor.matmul(
                    ksum_ps, sct_sb[:, 2 * t : 2 * t + 2], k_r,
                    start=t == 0, stop=t == NT - 1,
                )
                # kv: per d2-tile [128, 512] = [ (k*sin).T@v | (k*cos).T@v ]
                # The s-half and c-half share one PSUM bank (= one 2 KiB zero
                # region): the s-group's start clears the whole bank, so the
                # c-group never sets start, and only the c-group's final
                # matmul sets stop.
                for d2 in range(DT):
                    nc.tensor.matmul(
                        kv_ps[d2][:, 0:MH], k_s[:, ts(d2, 128)], v_sb,
                        start=t == 0, stop=False,
                    )
                    nc.tensor.matmul(
                        kv_ps[d2][:, MH:512], k_c[:, ts(d2, 128)], v_sb,
                        start=False, stop=t == NT - 1,
                    )

            # ---- move kv/ksum to SBUF; transpose ksum to [d2-part, dt, 2] ----
            kv_sb = consts.tile([128, DT, 512], MM_DT)
            for d2 in range(DT):
                nc.vector.tensor_copy(kv_sb[:, d2, :], kv_ps[d2])
            ksum_row = work.tile([2, 512], MM_DT, tag="ksrow")
            nc.vector.tensor_copy(ksum_row, ksum_ps)
            ksum_sb = consts.tile([128, DT, 2], MM_DT)
            for d2 in range(DT):
                # reuse the kv bank slots (free after the kv_sb copies)
                tp = kvpool.tile([128, 2], MM_DT, tag=f"kv{d2}", name=f"tp{d2}")
                nc.tensor.transpose(tp, ksum_row[:, ts(d2, 128)], id_sb[0:2, 0:2])
                nc.vector.tensor_copy(ksum_sb[:, d2, :], tp)

            # ---- output stage: per row tile, o-matmuls + tiny qs-matmuls ----
            for t in range(NT):
                sin_ap = sctf_sb[:, 2 * t : 2 * t + 1]
                cos_ap = sctf_sb[:, 2 * t + 1 : 2 * t + 2]
                qt = qc[t // 4]
                nslc = ts(t % 4, 128)

                o_ps = ppool.tile([128, 512], F32, tag="proj", name=f"o_ps{t}")
                for d2 in range(DT):
                    nc.tensor.matmul(
                        o_ps, qt[:, d2, nslc], kv_sb[:, d2, :],
                        start=d2 == 0, stop=d2 == DT - 1,
                    )
                qs_ps = kvpool.tile([128, 2], F32, tag=f"kv{t % 2}", name=f"qs_ps{t}")
                for d2 in range(DT):
                    nc.tensor.matmul(
                        qs_ps, qt[:, d2, nslc], ksum_sb[:, d2, :],
                        start=d2 == 0, stop=d2 == DT - 1,
                    )

                # z = 1 / max(sin*qs_s + cos*qs_c, 1e-6)
                qq = opool.tile([128, 2], F32, tag="qq", name=f"qq{t}")
                nc.vector.tensor_mul(qq, qs_ps, sctf_sb[:, 2 * t : 2 * t + 2])
                qs1 = opool.tile([128, 1], F32, tag="qs1", name=f"qs1{t}")
                nc.vector.tensor_reduce(qs1, qq, axis=mybir.AxisListType.X, op=ALU.add)
                zt = opool.tile([128, 1], F32, tag="zt", name=f"zt{t}")
                nc.vector.tensor_scalar_max(qs1, qs1, 1e-6)
                nc.vector.reciprocal(zt, qs1)
                sz = opool.tile([128, 1], F32, tag="szl", name=f"szl{t}")
                nc.vector.tensor_mul(sz, zt, sin_ap)
                cz = opool.tile([128, 1], F32, tag="czl", name=f"czl{t}")
                nc.vector.tensor_mul(cz, zt, cos_ap)

                # out = (sin*z)*o_s + (cos*z)*o_c, scrambled store
                resA = opool.tile([128, MH], F32, tag="resA", name=f"resA{t}")
                nc.scalar.activation(resA, o_ps[:, 0:MH], AF.Copy, scale=sz)
                resB = opool.tile([128, MH], F32, tag="resB", name=f"resB{t}")
                nc.vector.tensor_scalar_mul(resB, o_ps[:, MH:512], cz)
                res = opool.tile([128, MH], F32, tag="res", name=f"res{t}")
                nc.vector.tensor_add(res, resB, resA)

                h, s0 = t // 4, (t % 4) * 128
                nc.sync.dma_start(
                    out_r[ds(s0, 128), :, h, :],
                    res.rearrange("p (j hd) -> p j hd", hd=64),
                )

    nc.compile()
    return nc


_prog_cache = {}


def get_program():
    if "nc" not in _prog_cache:
        _prog_cache["nc"] = build_program()
    return _prog_cache["nc"]


def make_in_maps(x, Wq, Wk, Wv):
    x = np.ascontiguousarray(np.asarray(x, dtype=np.float32))
    Wq = np.asarray(Wq, dtype=np.float32)
    Wk = np.asarray(Wk, dtype=np.float32)
    Wv = np.asarray(Wv, dtype=np.float32)

    idx = (np.pi / 2) * np.arange(1, N + 1, dtype=np.float64) / N
    sin = np.sin(idx).astype(np.float32)
    cos = np.cos(idx).astype(np.float32)
    sct = np.empty((128, 2 * NT), dtype=np.float32)
    for t in range(NT):
        sct[:, 2 * t] = sin[t * 128 : (t + 1) * 128]
        sct[:, 2 * t + 1] = cos[t * 128 : (t + 1) * 128]
    ident = np.eye(128, dtype=np.float32)

    xT = np.ascontiguousarray(x.transpose(0, 2, 1))  # [B, D, N]
    WqT = np.ascontiguousarray(Wq.T)
    WkT = np.ascontiguousarray(Wk.T)
    WvT = np.ascontiguousarray(Wv.T)

    np_mm = ml_dtypes.bfloat16 if MM_DT == BF16 else np.float32
    in_maps = []
    for core in range(8):
        b, J = core >> 1, core & 1
        in_maps.append(
            {
                "xT": np.ascontiguousarray(xT[b], dtype=np_mm),
                "wqT": WqT.astype(np_mm),
                "wkT": WkT.astype(np_mm),
                "wvT": np.ascontiguousarray(
                    WvT[:, J * MH : (J + 1) * MH], dtype=np_mm
                ),
                "sct": sct.astype(np_mm),
                "sctf": sct,
                "ident": ident.astype(np_mm),
            }
        )
    return in_maps


def assemble(results):
    out = np.empty((B, N, D), dtype=np.float32)
    for b in range(B):
        b0 = results[2 * b]["out"].reshape(512, 4, 512)
        b1 = results[2 * b + 1]["out"].reshape(512, 4, 512)
        out[b] = np.stack([b0, b1], axis=1).reshape(N, D)
    return out


def run(x, Wq, Wk, Wv, **spmd_kwargs):
    nc = get_program()
    in_maps = make_in_maps(x, Wq, Wk, Wv)
    res = run_bass_kernel_spmd(nc, in_maps, list(range(8)), **spmd_kwargs)
    return assemble(res.results), res


def kernel(x, Wq, Wk, Wv):
    out, _ = run(x, Wq, Wk, Wv)
    return out


# revision 13
# speedup vs baseline: 1.0333x; 1.0333x over previous
"""Cosformer multi-head attention kernel for 8 Trainium2 NeuronCores.

Reference computation (per batch b):
    q = relu(x @ Wq.T); k = relu(x @ Wk.T); v = x @ Wv.T
    q_ = [q*sin, q*cos]; k_ = [k*sin, k*cos]      (sin/cos indexed by position n)
    kv = k_.T @ v;  z = 1/clip(q_ @ sum_n k_, 1e-6)
    out = (q_ @ kv) * z   followed by a head-scrambling reshape.

Sharding: 8 cores = 4 batches x 2 halves of the output columns (m). Each core
computes q and k in full for its batch (needed for the full-feature contraction)
and v / kv / out only for its 256-column half. No collectives needed.

Decomposition used on-chip (avoids materializing the 2D-wide q_/k_):
    kv_s = (k*sin).T @ v ; kv_c = (k*cos).T @ v            [512, 256] each
    ksum_s = sum_n (k*sin) ; ksum_c = sum_n (k*cos)        [512]
    o_s = q @ kv_s ; o_c = q @ kv_c ; qs_s = q @ ksum_s ; qs_c = q @ ksum_c
    out = (sin*o_s + cos*o_c) / clip(sin*qs_s + cos*qs_c, 1e-6)

All matmuls run as float32r (FP22 multiplies, fp32 accumulate) which is 4x
faster than true fp32 on the PE array at free-dim >= 256. The walrus verifier
requires every tensor consumed by an FP32r matmul to be declared float32r at
its producer, so all matmul operands are float32r end-to-end (bit-compatible
with fp32 on the host side).

The final head-scramble permutation is folded into the output DMA access
pattern; the host only does cheap reshapes to reassemble the full output.
"""

import math
import numpy as np
import ml_dtypes

import concourse.bass as bass
import concourse.mybir as mybir
import concourse.tile as tile
from concourse import bacc
from concourse.bass import ts, ds
from concourse.bass_utils import run_bass_kernel_spmd

B, N, D = 4, 4096, 512
MH = 256          # per-core output column half width
NT = N // 128     # 32 n-tiles
DT = D // 128     # 4 d-tiles
F32 = mybir.dt.float32
F32R = mybir.dt.float32r
BF16 = mybir.dt.bfloat16
import os
MM_DT = BF16 if os.environ.get("KERNEL_BF16") else F32R
AF = mybir.ActivationFunctionType
ALU = mybir.AluOpType


def f32(ap):
    """Read an f32r tensor as plain fp32 (same bits) for non-matmul ops."""
    return ap.bitcast(F32)


def build_program():
    nc = bacc.Bacc("TRN2", target_bir_lowering=False, debug=False, num_devices=8)

    xT = nc.dram_tensor("xT", [D, N], MM_DT, kind="ExternalInput").ap()
    wqT = nc.dram_tensor("wqT", [D, D], MM_DT, kind="ExternalInput").ap()
    wkT = nc.dram_tensor("wkT", [D, D], MM_DT, kind="ExternalInput").ap()
    wvT = nc.dram_tensor("wvT", [D, MH], MM_DT, kind="ExternalInput").ap()
    sct = nc.dram_tensor("sct", [128, 2 * NT], MM_DT, kind="ExternalInput").ap()
    sctf = nc.dram_tensor("sctf", [128, 2 * NT], F32, kind="ExternalInput").ap()
    ident = nc.dram_tensor("ident", [128, 128], MM_DT, kind="ExternalInput").ap()
    out = nc.dram_tensor("out", [N // 2, D], F32, kind="ExternalOutput").ap()

    # scrambled output view: tile rows rr = h*512 + s, cols cc = jj*64 + hd
    # land at out[s*4 + jj, h*64 + hd]
    out_r = out.rearrange("(s j) (h hd) -> s j h hd", j=4, hd=64)

    with tile.TileContext(nc) as tc:
        with (
            tc.tile_pool(name="consts", bufs=1) as consts,
            tc.tile_pool(name="xpool", bufs=1) as xpool,
            tc.tile_pool(name="qpool", bufs=1) as qpool,
            tc.tile_pool(name="work", bufs=2) as work,
            tc.tile_pool(name="opool", bufs=3) as opool,
            tc.tile_pool(name="ppool", bufs=3, space="PSUM") as ppool,
            tc.tile_pool(name="kvpool", bufs=1, space="PSUM") as kvpool,
        ):
            # ---- constants ----
            # Small/weight loads go on the ACT HWDGE ring, x chunks on the SP
            # ring, so the first k/v matmuls (need sct+wk+wv+xc0) start ASAP.
            sct_sb = consts.tile([128, 2 * NT], MM_DT)
            nc.scalar.dma_start(sct_sb, sct)
            sctf_sb = consts.tile([128, 2 * NT], F32)
            nc.scalar.dma_start(sctf_sb, sctf)
            wk_sb = consts.tile([128, DT, D], MM_DT)
            nc.scalar.dma_start(wk_sb, wkT.rearrange("(dt p) m -> p dt m", p=128))
            wv_sb = consts.tile([128, DT, MH], MM_DT)
            nc.scalar.dma_start(wv_sb, wvT.rearrange("(dt p) m -> p dt m", p=128))

            # ---- x, 8 chunks of [128, 4, 512] (1 MiB DMAs) ----
            xc = []
            xT_r = xT.rearrange("(dt p) n -> p dt n", p=128)
            for c in range(8):
                t_ = xpool.tile([128, DT, 512], MM_DT, tag=f"xc{c}", name=f"xc{c}")
                if c == 0:
                    # split the first chunk so the first k/v matmuls (which
                    # read only a 128-column slice) aren't gated on the full
                    # 1 MiB transfer + semaphore batching.
                    for s4 in range(4):
                        nc.sync.dma_start(
                            t_[:, :, ts(s4, 128)], xT_r[:, :, ts(s4, 128)]
                        )
                else:
                    nc.sync.dma_start(t_, xT_r[:, :, ts(c, 512)])
                xc.append(t_)

            wq_sb = consts.tile([128, DT, D], MM_DT)
            nc.scalar.dma_start(wq_sb, wqT.rearrange("(dt p) m -> p dt m", p=128))
            id_sb = consts.tile([128, 128], MM_DT)
            nc.scalar.dma_start(id_sb, ident)

            # qT storage [m-part, mt, n] chunks
            qc = [
                qpool.tile([128, DT, 512], MM_DT, tag=f"qc{c}", name=f"qc{c}")
                for c in range(8)
            ]

            # persistent psum accumulators
            kv_ps = [
                kvpool.tile([128, 512], F32, tag=f"kv{d2}", name=f"kv{d2}")
                for d2 in range(DT)
            ]
            ksum_ps = kvpool.tile([2, 512], F32, tag="ksum", name="ksum")

            # ---- fused loop: k/v/q projections + kv/ksum accumulation ----
            for t in range(NT):
                xt = xc[t // 4]
                nslc = ts(t % 4, 128)  # n-tile slice inside the x chunk
                sin_ap = sctf_sb[:, 2 * t : 2 * t + 1]
                cos_ap = sctf_sb[:, 2 * t + 1 : 2 * t + 2]

                k_ps = ppool.tile([128, 512], F32, tag="proj", name=f"k_ps{t}")
                for dt in range(DT):
                    nc.tensor.matmul(
                        k_ps, xt[:, dt, nslc], wk_sb[:, dt, :],
                        start=dt == 0, stop=dt == DT - 1,
                    )
                v_ps = ppool.tile([128, 512], F32, tag="proj", name=f"v_ps{t}")
                for dt in range(DT):
                    nc.tensor.matmul(
                        v_ps[:, :MH], xt[:, dt, nslc], wv_sb[:, dt, :],
                        start=dt == 0, stop=dt == DT - 1,
                    )
                mt, nc2 = t % 4, t // 4
                q_ps = ppool.tile([128, 512], F32, tag="proj", name=f"q_ps{t}")
                for dt in range(DT):
                    nc.tensor.matmul(
                        q_ps, wq_sb[:, dt, ts(mt, 128)], xc[nc2][:, dt, :],
                        start=dt == 0, stop=dt == DT - 1,
                    )
                nc.scalar.activation(qc[nc2][:, mt, :], q_ps, AF.Relu)

                k_s = work.tile([128, 512], MM_DT, tag="ks", name=f"ks{t}")
                nc.scalar.activation(k_s, k_ps, AF.Relu, scale=sin_ap)
                k_c = work.tile([128, 512], MM_DT, tag="kc", name=f"kc{t}")
                nc.scalar.activation(k_c, k_ps, AF.Relu, scale=cos_ap)
                k_r = work.tile([128, 512], MM_DT, tag="kr", name=f"kr{t}")
                nc.vector.tensor_scalar_max(k_r, k_ps, 0.0)
                v_sb = work.tile([128, MH], MM_DT, tag="vs", name=f"vs{t}")
                nc.vector.tensor_copy(v_sb, v_ps[:, :MH])

                # ksum: [2, 512] += sincos[128,2].T @ relu(k)[128,512]
                nc.tensor.matmul(
                    ksum_ps, sct_sb[:, 2 * t : 2 * t + 2], k_r,
                    start=t == 0, stop=t == NT - 1,
                )
                # kv: per d2-tile [128, 512] = [ (k*sin).T@v | (k*cos).T@v ]
                # The s-half and c-half share one PSUM bank (= one 2 KiB zero
                # region): the s-group's start clears the whole bank, so the
                # c-group never sets start, and only the c-group's final
                # matmul sets stop.
                for d2 in range(DT):
                    nc.tensor.matmul(
                        kv_ps[d2][:, 0:MH], k_s[:, ts(d2, 128)], v_sb,
                        start=t == 0, stop=False,
                    )
                    nc.tensor.matmul(
                        kv_ps[d2][:, MH:512], k_c[:, ts(d2, 128)], v_sb,
                        start=False, stop=t == NT - 1,
                    )

            # ---- move kv/ksum to SBUF; transpose ksum to [d2-part, dt, 2] ----
            kv_sb = consts.tile([128, DT, 512], MM_DT)
            for d2 in range(DT):
                nc.vector.tensor_copy(kv_sb[:, d2, :], kv_ps[d2])
            ksum_row = work.tile([2, 512], MM_DT, tag="ksrow")
            nc.vector.tensor_copy(ksum_row, ksum_ps)
            ksum_sb = consts.tile([128, DT, 2], MM_DT)
            for d2 in range(DT):
                # reuse the kv bank slots (free after the kv_sb copies)
                tp = kvpool.tile([128, 2], MM_DT, tag=f"kv{d2}", name=f"tp{d2}")
                nc.tensor.transpose(tp, ksum_row[:, ts(d2, 128)], id_sb[0:2, 0:2])
                nc.vector.tensor_copy(ksum_sb[:, d2, :], tp)

            # ---- output stage: per row tile, o-matmuls + tiny qs-matmuls ----
            for t in range(NT):
                sin_ap = sctf_sb[:, 2 * t : 2 * t + 1]
                cos_ap = sctf_sb[:, 2 * t + 1 : 2 * t + 2]
                qt = qc[t // 4]
                nslc = ts(t % 4, 128)

                o_ps = ppool.tile([128, 512], F32, tag="proj", name=f"o_ps{t}")
                for d2 in range(DT):
                    nc.tensor.matmul(
                        o_ps, qt[:, d2, nslc], kv_sb[:, d2, :],
                        start=d2 == 0, stop=d2 == DT - 1,
                    )
                qs_ps = kvpool.tile([128, 2], F32, tag=f"kv{t % 2}", name=f"qs_ps{t}")
                for d2 in range(DT):
                    nc.tensor.matmul(
                        qs_ps, qt[:, d2, nslc], ksum_sb[:, d2, :],
                        start=d2 == 0, stop=d2 == DT - 1,
                    )

                # z = 1 / max(sin*qs_s + cos*qs_c, 1e-6)
                qq = opool.tile([128, 2], F32, tag="qq", name=f"qq{t}")
                nc.vector.tensor_mul(qq, qs_ps, sctf_sb[:, 2 * t : 2 * t + 2])
                qs1 = opool.tile([128, 1], F32, tag="qs1", name=f"qs1{t}")
                nc.vector.tensor_reduce(qs1, qq, axis=mybir.AxisListType.X, op=ALU.add)
                zt = opool.tile([128, 1], F32, tag="zt", name=f"zt{t}")
                nc.vector.tensor_scalar_max(qs1, qs1, 1e-6)
                nc.vector.reciprocal(zt, qs1)
                sz = opool.tile([128, 1], F32, tag="szl", name=f"szl{t}")
                nc.vector.tensor_mul(sz, zt, sin_ap)
                cz = opool.tile([128, 1], F32, tag="czl", name=f"czl{t}")
                nc.vector.tensor_mul(cz, zt, cos_ap)

                # out = (sin*z)*o_s + (cos*z)*o_c, scrambled store
                resA = opool.tile([128, MH], F32, tag="resA", name=f"resA{t}")
                nc.scalar.activation(resA, o_ps[:, 0:MH], AF.Copy, scale=sz)
                resB = opool.tile([128, MH], F32, tag="resB", name=f"resB{t}")
                nc.vector.tensor_scalar_mul(resB, o_ps[:, MH:512], cz)
                res = opool.tile([128, MH], F32, tag="res", name=f"res{t}")
                nc.vector.tensor_add(res, resB, resA)

                h, s0 = t // 4, (t % 4) * 128
                nc.sync.dma_start(
                    out_r[ds(s0, 128), :, h, :],
                    res.rearrange("p (j hd) -> p j hd", hd=64),
                )

    nc.compile()
    return nc


_prog_cache = {}


def get_program():
    if "nc" not in _prog_cache:
        _prog_cache["nc"] = build_program()
    return _prog_cache["nc"]


def make_in_maps(x, Wq, Wk, Wv):
    x = np.ascontiguousarray(np.asarray(x, dtype=np.float32))
    Wq = np.asarray(Wq, dtype=np.float32)
    Wk = np.asarray(Wk, dtype=np.float32)
    Wv = np.asarray(Wv, dtype=np.float32)

    idx = (np.pi / 2) * np.arange(1, N + 1, dtype=np.float64) / N
    sin = np.sin(idx).astype(np.float32)
    cos = np.cos(idx).astype(np.float32)
    sct = np.empty((128, 2 * NT), dtype=np.float32)
    for t in range(NT):
        sct[:, 2 * t] = sin[t * 128 : (t + 1) * 128]
        sct[:, 2 * t + 1] = cos[t * 128 : (t + 1) * 128]
    ident = np.eye(128, dtype=np.float32)

    xT = np.ascontiguousarray(x.transpose(0, 2, 1))  # [B, D, N]
    WqT = np.ascontiguousarray(Wq.T)
    WkT = np.ascontiguousarray(Wk.T)
    WvT = np.ascontiguousarray(Wv.T)

    np_mm = ml_dtypes.bfloat16 if MM_DT == BF16 else np.float32
    in_maps = []
    for core in range(8):
        b, J = core >> 1, core & 1
        in_maps.append(
            {
                "xT": np.ascontiguousarray(xT[b], dtype=np_mm),
                "wqT": WqT.astype(np_mm),
                "wkT": WkT.astype(np_mm),
                "wvT": np.ascontiguousarray(
                    WvT[:, J * MH : (J + 1) * MH], dtype=np_mm
                ),
                "sct": sct.astype(np_mm),
                "sctf": sct,
                "ident": ident.astype(np_mm),
            }
        )
    return in_maps


def assemble(results):
    out = np.empty((B, N, D), dtype=np.float32)
    for b in range(B):
        b0 = results[2 * b]["out"].reshape(512, 4, 512)
        b1 = results[2 * b + 1]["out"].reshape(512, 4, 512)
        out[b] = np.stack([b0, b1], axis=1).reshape(N, D)
    return out


def run(x, Wq, Wk, Wv, **spmd_kwargs):
    nc = get_program()
    in_maps = make_in_maps(x, Wq, Wk, Wv)
    res = run_bass_kernel_spmd(nc, in_maps, list(range(8)), **spmd_kwargs)
    return assemble(res.results), res


def kernel(x, Wq, Wk, Wv):
    out, _ = run(x, Wq, Wk, Wv)
    return out
